# revision 62
# baseline (speedup 1.0000x reference)
"""Trainium2 Bass kernel for nn_Attention_31215822307478.

EfficientViT-style attention block:
  qkv 1x1 conv + BN -> split q,k,v -> depthwise 3x3 + BN on q ->
  8-head attention with positional bias over N=784 tokens ->
  ReLU -> 1x1 proj + BN.

Strategy (per core, data-parallel over batch, 4 images/core):
  - BN folded into conv weights/biases on host.
  - q/k produced in head-padded layout with each 16-ch head DUPLICATED into
    a full 32-aligned partition slot (K=32 contraction, exp scale halved) so
    the QK^T matmuls keep half the PE rows streaming and row-tiled pairs of
    heads run concurrently (2x) -- keeps the PE HAM activity monitor from
    throttling the clock to 1.2 GHz during attention (the baseline's main
    loss: 467us of the 588us span ran at half clock).
  - AV is col-tiled: the two heads of a pair run concurrently at
    tile_position (0,0)/(0,64) with M=64 each -> full-array activity.
  - The softmax denominator Z comes from ones-weight M=64 matmuls that
    broadcast Z across the pair's 128 output partitions directly, so the
    division needs no cross-partition gather or PE broadcast afterwards:
    one reciprocal_approx_fast + one multiply + one fused bias+relu.
  - The conv V bias is folded past the (linear) AV and division:
    relu(U/Z + bv) with bv applied as a per-partition scalar.
  - Depthwise 3x3 runs on the PE as 9 accumulated diagonal matmuls over a
    zero-padded [128, 30x30] buffer.
  - Softmax is max-free (logits provably small); positional bias enters as a
    host-precomputed exp(bias) table (bf16) via one elementwise multiply,
    split between the DVE and GpSimd engines; exp is the Scalar engine's
    only job (it is the throughput wall at ~53us/image).
  - Conv chunks of image b+1 are interleaved between the attention head-pair
    blocks of image b to fill PE gaps while ACT chews through the exps.
"""

import os
import sys

import numpy as np

for _p in ("/opt/trn_rl_repo", "/root/.axon_site/_ro/trn_rl_repo"):
    if os.path.isdir(_p) and _p not in sys.path:
        sys.path.insert(0, _p)

import ml_dtypes  # noqa: E402
from contextlib import ExitStack  # noqa: E402

import concourse.bass as bass  # noqa: E402
import concourse.mybir as mybir  # noqa: E402
import concourse.tile as tile  # noqa: E402
from concourse import bacc  # noqa: E402
from concourse.alu_op_type import AluOpType  # noqa: E402
from concourse.bass_utils import run_bass_kernel_spmd  # noqa: E402

EPS = 1e-5
DIM, KEY_DIM, HEADS = 256, 16, 8
NH_KD, D, DH = 128, 64, 512
B, H, W = 32, 28, 28
N = H * W  # 784
NCORES = 8
BC = B // NCORES  # 4 images per core
SCALE = KEY_DIM ** -0.5
HSCALE = SCALE / 2.0  # duplicated q/k rows double the dot product

F32 = mybir.dt.float32
BF16 = mybir.dt.bfloat16
AF = mybir.ActivationFunctionType

KT_SIZES = [128] * 6 + [16]  # 784 = 6*128 + 16 key tiles
CH = [(0, 512), (512, 272)]  # PSUM-bank-aligned free chunks of 784

_PROGRAM_CACHE = {}


def _build_program():
    nc = bacc.Bacc("TRN2", target_bir_lowering=False, debug=False)

    x_d = nc.dram_tensor("x", [BC, 256, N], BF16, kind="ExternalInput").ap()
    wqkT_d = nc.dram_tensor("wqkT", [2, 128, 512], BF16, kind="ExternalInput").ap()
    wvT_d = nc.dram_tensor("wvT", [2, 128, 512], BF16, kind="ExternalInput").ap()
    wpT_d = nc.dram_tensor("wpT", [4, 128, 256], BF16, kind="ExternalInput").ap()
    dtap_d = nc.dram_tensor("dtaps", [2, 128, 9 * 128], BF16, kind="ExternalInput").ap()
    bias_d = nc.dram_tensor("biases", [128, 12], F32, kind="ExternalInput").ap()
    eb_d = nc.dram_tensor("eb", [128, HEADS * 7 * N], BF16, kind="ExternalInput").ap()
    ones_d = nc.dram_tensor("onesw", [128, 64], BF16, kind="ExternalInput").ap()
    out_d = nc.dram_tensor("out", [BC, 256, N], F32, kind="ExternalOutput").ap()

    with tile.TileContext(nc) as tc, ExitStack() as ctx:
        const = ctx.enter_context(tc.tile_pool(name="const", bufs=1))
        # PSUM: S/conv/proj pool 3x2 banks + 1-bank U/Z chunk tiles (2 bufs)
        pspool = ctx.enter_context(tc.tile_pool(name="ps", bufs=3, space="PSUM"))
        uzpool = ctx.enter_context(tc.tile_pool(name="uz", bufs=2, space="PSUM"))
        xpool = ctx.enter_context(tc.tile_pool(name="xp", bufs=3))
        qpadp = ctx.enter_context(tc.tile_pool(name="qpadp", bufs=3))
        kpool = ctx.enter_context(tc.tile_pool(name="kp", bufs=5))
        qdpool = ctx.enter_context(tc.tile_pool(name="qdp", bufs=5))
        vpool = ctx.enter_context(tc.tile_pool(name="vp", bufs=14))
        epool = ctx.enter_context(tc.tile_pool(name="ep", bufs=6))
        apool = ctx.enter_context(tc.tile_pool(name="ap", bufs=22))
        rzpool = ctx.enter_context(tc.tile_pool(name="rzp", bufs=2))
        tdpool = ctx.enter_context(tc.tile_pool(name="tdp", bufs=2))
        rtpool = ctx.enter_context(tc.tile_pool(name="rtp", bufs=6))
        opool = ctx.enter_context(tc.tile_pool(name="op", bufs=2))

        # ---- constants (tiles created here; DMAs issued after image 0's
        # x-load trigger so x isn't stuck behind ~12 serial queue triggers) --
        wqkT = []
        wvT = []
        wpT = []
        dtap = []
        for ki in range(2):
            t = const.tile([128, 512], BF16, tag=f"wqkT{ki}", name=f"wqkT{ki}")
            wqkT.append(t)
        for ki in range(2):
            t = const.tile([128, 512], BF16, tag=f"wvT{ki}", name=f"wvT{ki}")
            wvT.append(t)
        for hp in range(4):
            t = const.tile([128, 256], BF16, tag=f"wpT{hp}", name=f"wpT{hp}")
            wpT.append(t)
        for g in range(2):
            t = const.tile([128, 9 * 128], BF16, tag=f"dtap{g}", name=f"dtap{g}")
            dtap.append(t)
        biases = const.tile([128, 12], F32, tag="biases", name="biases")
        onesw = const.tile([128, 64], BF16, tag="onesw", name="onesw")
        eb = const.tile([128, HEADS * 7 * N], BF16, tag="eb", name="eb")

        def load_consts():
            for ki in range(2):
                nc.sync.dma_start(wqkT[ki][:], wqkT_d[ki])
            for g in range(2):
                nc.sync.dma_start(dtap[g][:], dtap_d[g])
            nc.sync.dma_start(biases[:], bias_d[:])
            for ki in range(2):
                nc.sync.dma_start(wvT[ki][:], wvT_d[ki])
            nc.sync.dma_start(onesw[:], ones_d[:])
            for hp in range(4):
                nc.sync.dma_start(wpT[hp][:], wpT_d[hp])

        st = [dict() for _ in range(BC)]  # per-image tile state

        def item_xload(b):
            s = st[b]
            s["xb"] = []
            for ki in range(2):
                t = xpool.tile([128, N], BF16, tag="xb", name="xb")
                nc.sync.dma_start(t[:], x_d[b, 128 * ki : 128 * (ki + 1), :])
                s["xb"].append(t)

        def item_qk(b, mt):
            # one output tile of the qk 1x1 conv (duplicated head-padded)
            s = st[b]
            ps = pspool.tile([128, N], F32, tag="ps", name="ps")
            for (o, szc) in CH:
                for ki in range(2):
                    nc.tensor.matmul(
                        ps[:, o : o + szc],
                        wqkT[ki][:, mt * 128 : (mt + 1) * 128],
                        s["xb"][ki][:, o : o + szc],
                        start=(ki == 0),
                        stop=(ki == 1),
                    )
            if mt < 2:
                qp = qpadp.tile([128, 900], BF16, tag="qpad", name="qpad")
                nc.gpsimd.memset(qp[:], 0.0)
                qp3 = qp[:].rearrange("p (y x) -> p y x", y=30)
                nc.vector.tensor_scalar_add(
                    qp3[:, 1:29, 1:29],
                    ps[:].rearrange("p (y x) -> p y x", y=28),
                    biases[:, mt : mt + 1],
                )
                s.setdefault("q_pad", []).append(qp)
            else:
                kt_ = kpool.tile([128, N], BF16, tag="ksb", name="ksb")
                nc.vector.tensor_scalar_add(kt_[:], ps[:], biases[:, mt : mt + 1])
                s.setdefault("k_sb", []).append(kt_)

        def item_v(b, sp):
            # one spatial tile of the v 1x1 conv, transposed [spatial, 8*64]
            s = st[b]
            ssz = KT_SIZES[sp]
            psv = pspool.tile([128, 512], F32, tag="ps", name="ps")
            for ki in range(2):
                nc.tensor.matmul(
                    psv[:ssz, :],
                    s["xb"][ki][:, sp * 128 : sp * 128 + ssz],
                    wvT[ki][:, :],
                    start=(ki == 0),
                    stop=(ki == 1),
                )
            vtt = vpool.tile([128, 512], BF16, tag="vt", name="vt")
            nc.vector.tensor_copy(vtt[:ssz, :], psv[:ssz, :])
            s.setdefault("vt", []).append(vtt)

        def item_dw(b, g, half):
            # depthwise 3x3 via 9 diagonal matmuls, one 14-row half
            s = st[b]
            s.setdefault("dwh", [0, 0])[g] += 1
            y0 = 14 * half
            psd = pspool.tile([128, 512], F32, tag="ps", name="ps")
            qp3 = s["q_pad"][g][:].rearrange("p (y x) -> p y x", y=30)
            for t9 in range(9):
                ty, tx = divmod(t9, 3)
                nc.tensor.matmul(
                    psd[:, 0 : 14 * 28],
                    dtap[g][:, t9 * 128 : (t9 + 1) * 128],
                    qp3[:, y0 + ty : y0 + ty + 14, tx : tx + 28],
                    start=(t9 == 0),
                    stop=(t9 == 8),
                )
            if half == 0:
                qd = qdpool.tile([128, N], BF16, tag="qdw", name="qdw")
                s.setdefault("q_dw", []).append(qd)
            qd = s["q_dw"][g]
            nc.vector.tensor_scalar_add(
                qd[:, 392 * half : 392 * half + 392],
                psd[:, 0:392],
                biases[:, 4 + g : 5 + g],
            )

        def phase_a_items(b):
            # conv work for image b as ~1us micro-items, interleaved into the
            # previous image's attention stream to keep the PE dense; head
            # group 0's prerequisites (q/k tiles + their depthwise) come
            # first so attention on the pair (0,1) can start early
            items = [(100, lambda b=b: item_xload(b))]
            items.append((900, lambda b=b: item_qk(b, 0)))
            items.append((900, lambda b=b: item_qk(b, 2)))
            items.append((900, lambda b=b: item_dw(b, 0, 0)))
            items.append((900, lambda b=b: item_dw(b, 0, 1)))
            for sp in range(7):
                items.append((900, lambda b=b, sp=sp: item_v(b, sp)))
            items.append((900, lambda b=b: item_qk(b, 1)))
            items.append((900, lambda b=b: item_qk(b, 3)))
            items.append((900, lambda b=b: item_dw(b, 1, 0)))
            items.append((900, lambda b=b: item_dw(b, 1, 1)))
            return items

        from collections import deque

        fill = deque()  # (pe_cost_ns, closure) of deferred PE/DVE work

        def pull(budget):
            while budget > 0 and fill:
                cost, it = fill.popleft()
                it()
                budget -= max(cost, 1)

        def phase_qk(b, hp):
            # QK^T + exp + eb-mult for head pair (2hp, 2hp+1); the S pool's
            # 3 buffers let the PE run ~1.5 kt ahead of the Scalar engine so
            # exps go back-to-back; deferred work is pulled into the PE's
            # idle window each iteration
            s = st[b]
            g = hp // 2
            jA = (2 * hp) % 4
            jB = jA + 1
            Alist = []
            for kt in range(7):
                ksz = KT_SIZES[kt]
                Ss = []
                # QK^T: two heads row-tiled concurrent (K=32 each)
                for (o, szc) in CH:
                    for hi, j in enumerate((jA, jB)):
                        if len(Ss) < 2:
                            Ss.append(pspool.tile([128, N], F32, tag="ps", name="ps"))
                        nc.tensor.matmul(
                            Ss[hi][:ksz, o : o + szc],
                            s["k_sb"][g][32 * j : 32 * j + 32, kt * 128 : kt * 128 + ksz],
                            s["q_dw"][g][32 * j : 32 * j + 32, o : o + szc],
                            start=True,
                            stop=True,
                            tile_position=(32 * j, 0),
                        )
                pull(2400 if (hp == 3 or b == BC - 1) else 1300)
                As = []
                for hi, h in enumerate((2 * hp, 2 * hp + 1)):
                    E = epool.tile([128, N], BF16, tag="E", name="E")
                    nc.scalar.activation(E[:ksz, :], Ss[hi][:ksz, :], AF.Exp, scale=HSCALE)
                    A = apool.tile([128, N], BF16, tag="A", name="A")
                    eng = nc.gpsimd if (hi and kt < 5) else nc.vector
                    eng.tensor_tensor(
                        A[:ksz, :],
                        E[:ksz, :],
                        eb[:ksz, (h * 7 + kt) * N : (h * 7 + kt + 1) * N],
                        op=AluOpType.mult,
                    )
                    As.append(A)
                Alist.append(As)
            return Alist

        def queue_avz(b, hp, Alist, conv_items):
            # deferred per-q-chunk AV + Z-broadcast accumulation over kt,
            # then division; U/Z are 1-bank PSUM tiles. Conv micro-items are
            # placed between the division and the next chunk block so the
            # U/Z buffer reuse never head-of-line blocks the PE on the DVE
            # division latency.
            s = st[b]
            hA = 2 * hp
            hB = hA + 1
            rt = rtpool.tile([128, N], BF16, tag="rt", name="rt")
            s.setdefault("rt", []).append(rt)
            for (o, szc) in CH:
                d = {}

                def mk_avz(kt, o=o, szc=szc, d=d):
                    def run():
                        if kt == 0:
                            d["U"] = uzpool.tile([128, 512], F32, tag="uz", name="uz")
                            d["Z"] = uzpool.tile([128, 512], F32, tag="uz", name="uz")
                        ksz = KT_SIZES[kt]
                        As = Alist[kt]
                        for hi, h in enumerate((hA, hB)):
                            nc.tensor.matmul(
                                d["U"][64 * hi : 64 * hi + 64, 0:szc],
                                s["vt"][kt][:ksz, 64 * h : 64 * h + 64],
                                As[hi][:ksz, o : o + szc],
                                start=(kt == 0),
                                stop=(kt == 6),
                                tile_position=(0, 64 * hi),
                            )
                        for hi in range(2):
                            nc.tensor.matmul(
                                d["Z"][64 * hi : 64 * hi + 64, 0:szc],
                                onesw[:ksz, :],
                                As[hi][:ksz, o : o + szc],
                                start=(kt == 0),
                                stop=(kt == 6),
                                tile_position=(0, 64 * hi),
                            )
                    return run

                def mk_div(o=o, szc=szc, d=d):
                    def run():
                        rz = rzpool.tile([128, 512], F32, tag="rz", name="rz")
                        nc.vector.reciprocal_approx_fast(rz[:, 0:szc], d["Z"][:, 0:szc])
                        td = tdpool.tile([128, 512], BF16, tag="td", name="td")
                        nc.vector.tensor_tensor(
                            td[:, 0:szc], d["U"][:, 0:szc], rz[:, 0:szc],
                            op=AluOpType.mult,
                        )
                        nc.vector.tensor_scalar(
                            rt[:, o : o + szc], td[:, 0:szc],
                            biases[:, 8 + hp : 9 + hp], 0.0,
                            op0=AluOpType.add, op1=AluOpType.max,
                        )
                    return run

                for kt in range(7):
                    fill.append((430 if szc == 512 else 230, mk_avz(kt)))
                fill.append((0, mk_div()))
                for _ in range(3):
                    if conv_items:
                        fill.append(conv_items.popleft())

        def phase_c(b):
            s = st[b]
            for mt in range(2):

                def mk_proj(mt=mt):
                    def run():
                        po_ = pspool.tile([128, N], F32, tag="ps", name="ps")
                        for (o, szc) in CH:
                            for hp in range(4):
                                nc.tensor.matmul(
                                    po_[:, o : o + szc],
                                    wpT[hp][:, mt * 128 : (mt + 1) * 128],
                                    s["rt"][hp][:, o : o + szc],
                                    start=(hp == 0),
                                    stop=(hp == 3),
                                )
                        ob = opool.tile([128, N], F32, tag="ob", name="ob")
                        nc.vector.tensor_scalar_add(
                            ob[:], po_[:], biases[:, 6 + mt : 7 + mt]
                        )
                        nc.sync.dma_start(out_d[b, mt * 128 : (mt + 1) * 128, :], ob[:])
                    return run

                fill.append((2700, mk_proj()))

        # image 0's convs go first so the PE has work while eb streams in
        # (eb DMAs ride the gpsimd queue so x/weight loads aren't stuck
        # behind 10MB on the sync queue)
        items0 = phase_a_items(0)
        items0[0][1]()  # x-load trigger first
        load_consts()
        for _c, it in items0[1:5]:
            it()
        for kt in range(7):
            nc.gpsimd.dma_start(
                eb[:, kt * N : (kt + 1) * N],
                eb_d[:, kt * N : (kt + 1) * N],
            )
        for h in range(1, HEADS):
            nc.sync.dma_start(
                eb[:, h * 7 * N : (h + 1) * 7 * N],
                eb_d[:, h * 7 * N : (h + 1) * 7 * N],
            )
        fill.extend(items0[5:])

        def ready_hp(b, hp):
            s = st[b]
            g = hp // 2
            return len(s.get("k_sb", [])) > g and s.get("dwh", [0, 0])[g] == 2

        for b in range(BC):
            conv_items = deque(phase_a_items(b + 1)) if b + 1 < BC else deque()
            for hp in range(4):
                # image b's conv items flow through `fill`; force stragglers
                # out before build-time references to their tiles
                while fill and not ready_hp(b, hp):
                    pull(4000)
                Alist = phase_qk(b, hp)
                queue_avz(b, hp, Alist, conv_items)
            while conv_items:
                fill.append(conv_items.popleft())
            phase_c(b)
        pull(1 << 30)

    nc.compile()
    return nc


def get_program():
    if "nc" not in _PROGRAM_CACHE:
        _PROGRAM_CACHE["nc"] = _build_program()
    return _PROGRAM_CACHE["nc"]


def prep_host_inputs(inputs):
    """Fold BN, reorder/duplicate/pad weights, build exp-bias table. Returns
    dict of np arrays for the non-x DRAM tensors (shared across cores)."""
    f32 = np.float32
    bf = ml_dtypes.bfloat16
    qkv_w = np.asarray(inputs["qkv_w"], f32)[:, :, 0, 0]  # [768, 256]
    s = np.asarray(inputs["qkv_g"], f32) / np.sqrt(np.asarray(inputs["qkv_v"], f32) + EPS)
    Wall = qkv_w * s[:, None]
    ball = np.asarray(inputs["qkv_b"], f32) - np.asarray(inputs["qkv_m"], f32) * s
    Wq, Wk, Wv = Wall[:128], Wall[128:256], Wall[256:]
    bq, bk, bv = ball[:128], ball[128:256], ball[256:]

    # q/k head-padded layout with each 16-row head duplicated to fill its
    # 32-aligned slot (doubles the dot product; exp scale is halved)
    qk_pad = np.zeros((512, 256), f32)
    bqk_pad = np.zeros(512, f32)
    for h in range(HEADS):
        g, j = divmod(h, 4)
        for rep in range(2):
            r0 = 128 * g + 32 * j + 16 * rep
            qk_pad[r0 : r0 + 16] = Wq[16 * h : 16 * h + 16]
            bqk_pad[r0 : r0 + 16] = bq[16 * h : 16 * h + 16]
            r1 = 128 * (2 + g) + 32 * j + 16 * rep
            qk_pad[r1 : r1 + 16] = Wk[16 * h : 16 * h + 16]
            bqk_pad[r1 : r1 + 16] = bk[16 * h : 16 * h + 16]
    wqkT = np.ascontiguousarray(qk_pad.T).reshape(2, 128, 512)

    # v: plain transposed weights, head-major columns, no bias (folded into
    # the post-division per-partition bias)
    wvT = np.ascontiguousarray(Wv.T).reshape(2, 128, 512)

    # depthwise taps: diagonal per-channel, duplicated rows get the same taps
    s2 = np.asarray(inputs["dw_g"], f32) / np.sqrt(np.asarray(inputs["dw_v"], f32) + EPS)
    dww = np.asarray(inputs["dw_w"], f32)[:, 0] * s2[:, None, None]  # [128,3,3]
    bdw = np.asarray(inputs["dw_b"], f32) - np.asarray(inputs["dw_m"], f32) * s2
    dtaps = np.zeros((2, 128, 9 * 128), f32)
    bdw_pad = np.zeros((2, 128), f32)
    for g in range(2):
        for p in range(128):  # p = output partition (normal layout)
            j, r = divmod(p, 32)
            c = 16 * (4 * g + j) + (r % 16)
            for t9 in range(9):
                dtaps[g, p, t9 * 128 + p] = dww[c].reshape(9)[t9]
            bdw_pad[g, p] = bdw[c]
    dtaps = dtaps.astype(bf)

    sp = np.asarray(inputs["proj_g"], f32) / np.sqrt(
        np.asarray(inputs["proj_v"], f32) + EPS
    )
    Wp = np.asarray(inputs["proj_w"], f32)[:, :, 0, 0] * sp[:, None]  # [256, 512]
    bp = np.asarray(inputs["proj_b"], f32) - np.asarray(inputs["proj_m"], f32) * sp
    wpT = np.stack(
        [np.ascontiguousarray(Wp[:, 128 * hp : 128 * hp + 128].T) for hp in range(4)]
    )  # [4, 128, 256]

    biases = np.zeros((128, 12), f32)
    biases[:, 0] = bqk_pad[0:128]
    biases[:, 1] = bqk_pad[128:256]
    biases[:, 2] = bqk_pad[256:384]
    biases[:, 3] = bqk_pad[384:512]
    biases[:, 4] = bdw_pad[0]
    biases[:, 5] = bdw_pad[1]
    biases[:, 6] = bp[:128]
    biases[:, 7] = bp[128:]
    for hp in range(4):
        biases[:, 8 + hp] = bv[128 * hp : 128 * hp + 128]

    ab = np.asarray(inputs["ab"], f32)  # [8, 784]
    idx = np.asarray(inputs["bias_idxs"])  # [784, 784] int32
    ebt = np.exp(ab)[:, idx]  # [8, 784(key), 784(q)] (bias is symmetric)
    eb = np.zeros((128, HEADS * 7 * N), f32)
    for h in range(HEADS):
        for kt in range(7):
            ksz = KT_SIZES[kt]
            blk = ebt[h, kt * 128 : kt * 128 + ksz, :]
            eb[:ksz, (h * 7 + kt) * N : (h * 7 + kt + 1) * N] = blk
    eb = eb.astype(bf)

    return {
        "onesw": np.ones((128, 64), np.float32).astype(bf),
        "wqkT": wqkT.astype(bf),
        "wvT": wvT.astype(bf),
        "wpT": wpT.astype(bf),
        "dtaps": dtaps,
        "biases": biases,
        "eb": eb,
    }


def kernel(**inputs):
    nc = get_program()
    shared = prep_host_inputs(inputs)
    x = np.asarray(inputs["x"], np.float32).reshape(B, 256, N).astype(ml_dtypes.bfloat16)
    in_maps = []
    for c in range(NCORES):
        m = dict(shared)
        m["x"] = np.ascontiguousarray(x[BC * c : BC * (c + 1)])
        in_maps.append(m)
    res = run_bass_kernel_spmd(nc, in_maps, core_ids=list(range(NCORES)))
    out = np.concatenate([r["out"] for r in res.results], axis=0)
    return out.reshape(B, 256, H, W)
